# revision 7
# baseline (speedup 1.0000x reference)
"""Trainium2 Bass kernel for CRF NLL loss (nn_CRF_71571335021248).

Strategy
--------
Data-parallel over batch B=128 across 8 cores (16 sequences per core).

The forward-algorithm logsumexp scan is reformulated in exp space:
    sigma_t = (E^T sigma_{t-1}) * e_t          E = exp(trans), e_t = exp(x_t)
Host-side we subtract the per-(b,t) logsumexp of the emissions (the NLL is
invariant), so fp32/bf16 never overflow and no renormalization is needed.

Latency wall of the sequential scan is broken with K parallel chains per
core exploiting Perron-Frobenius contraction: the per-step map
x -> (E^T x) * e_t is a positive linear map whose matrix products contract
direction exponentially (diagonal scalings are Hilbert-metric isometries;
tau = tanh(Delta(E)/4) ~ 0.27 here). A chain seeded with ones at position
p - W has the exact direction at p up to tau^W (~1e-7 for W=12). Each chain
covers one T-segment; relative scales are recovered host-side from component
ratios at the segment overlap points; everything combines in float64 on the
host from tiny [96,16] exported state vectors.

All chains run FORWARD, so the single stationary E is loaded once and there
are ZERO ldweights in steady state. Chains are ganged: ONE matmul advances a
whole gang (moving operand [96, 16*g]) and ONE DVE tensor_mul evacuates the
gang's PSUM block (amortizing the ~125 ns DVE PSUM-access init), multiplying
by emission columns the host pre-gathered into a step-major stream tensor
[96, D, 16*K] (exp'd, bf16). Sequential depth drops from 511 (bidirectional
baseline) to D ~ 55 steps.

Chain boundaries are aligned so ALL warmup snapshots land on step W-1 and
all terminals on step D-1 (slack absorbed by chain 0, whose terminal is the
only straggler): exports are 2 gang-wide [96,16g] copies per gang plus one
[96,16] copy, instead of per-chain copies that would serialize the DVE.

The gold-path score (point + transition gathers) is computed host-side.
"""

import math

import numpy as np

B, L = 128, 96
T_FULL = 1024
N_CORES = 8
BL = B // N_CORES  # 16 sequences per core

# chain config
K_CHAINS = 30
N_GANGS = 3
W_WARM = 10
CHUNK0 = 2  # steps in first DMA chunk (fast start)
CHUNK_SZ = 8

_PROGRAM_CACHE: dict = {}


def plan(K=K_CHAINS, W=W_WARM, T=T_FULL):
    """Aligned chain layout: snapshots all at step W-1, terminals at D-1.

    chain 0: seeded with e_0, absorbs positions 1..D, useful span [0, b1-1],
             terminal at position b1-1 = step b1-2.
    chain c>=1: seeded with ones at position b1+(c-1)(D-W)-W, useful span of
             D-W positions, snapshot at step W-1, terminal at step D-1.
    """
    D = math.ceil((T - 1 + (K - 1) * W) / K)
    b1 = T - (K - 1) * (D - W)
    assert W + 1 <= b1 <= D + 1, (K, W, D, b1)
    starts = [1] + [b1 + (c - 1) * (D - W) - W for c in range(1, K)]
    term_step = [b1 - 2] + [D - 1] * (K - 1)
    snap_step = [None] + [W - 1] * (K - 1)
    for c in range(1, K):
        assert starts[c - 1] + term_step[c - 1] == starts[c] + snap_step[c]
    assert starts[K - 1] + D - 1 == T - 1
    return D, starts, term_step, snap_step


def _chunks(D):
    """gp-queue DMA chunks (graduated sizes); steps [0,2) go via the SP queue."""
    out = []
    k = 2
    for sz in (4, 6):
        out.append((k, min(k + sz, D)))
        k += sz
    while k < D:
        out.append((k, min(k + CHUNK_SZ, D)))
        k += CHUNK_SZ
    return out


def _build_program(K=K_CHAINS, NG=N_GANGS, W=W_WARM):
    from contextlib import ExitStack

    import concourse.bass as bass
    from concourse import mybir

    f32 = mybir.dt.float32
    bf16 = mybir.dt.bfloat16
    assert K % NG == 0
    g = K // NG
    GW = 16 * g  # gang width in columns
    D, starts, term_step, snap_step = plan(K, W)
    chunks = _chunks(D)
    # export layout: [terms 16K | snaps 16K | chain0 term 16]
    NEXP = 2 * K + 1

    nc = bass.Bass()
    xg = nc.dram_tensor("xg", [L, D, 16 * K], bf16, kind="ExternalInput")
    wconst = nc.dram_tensor("wconst", [L, L + 16 * K], bf16, kind="ExternalInput")
    out = nc.dram_tensor("out", [L, 16 * NEXP], bf16, kind="ExternalOutput")

    es = ExitStack()
    with es:
        sem = lambda name: es.enter_context(nc.semaphore(name))
        sbuf = lambda name, shape, dt: es.enter_context(nc.sbuf_tensor(name, shape, dt))
        psum = lambda name, shape: es.enter_context(nc.psum_tensor(name, shape, f32))

        dma_m = sem("dma_m")
        dma_x = sem("dma_x")
        s_x0 = sem("s_x0")
        s_snap = sem("s_snap")
        s_pe = [sem(f"s_pe{G}") for G in range(NG)]
        s_dv = [sem(f"s_dv{G}") for G in range(NG)]
        s_fin = sem("s_fin")

        WC = sbuf("WC", [L, L + 16 * K], bf16)
        XG = sbuf("XG", [L, D, 16 * K], bf16)
        SIG = [sbuf("SIG0", [L, 16 * K], bf16), sbuf("SIG1", [L, 16 * K], bf16)]
        EXPB = sbuf("EXPB", [L, 16 * NEXP], bf16)
        DUM = sbuf("DUM", [1, 16], bf16)

        PS = [psum(f"PS{G}", [L, GW]) for G in range(NG)]

        chunk_start = {k0: ci for ci, (k0, k1) in enumerate(chunks)}

        with nc.Block() as block:

            @block.sync
            def _(sp):
                sp.dma_start(out=WC[:, 0:L], in_=wconst[:, 0:L]).then_inc(dma_m, 16)
                sp.dma_start(
                    out=WC[:, L:], in_=wconst[:, L:]
                ).then_inc(dma_m, 16)
                sp.dma_start(out=XG[:, 0:2, :], in_=xg[:, 0:2, :]).then_inc(s_x0, 16)
                sp.wait_ge(dma_m, 80)

            @block.gpsimd
            def _(gp):
                for (k0, k1) in chunks:
                    gp.dma_start(
                        out=XG[:, k0:k1, :], in_=xg[:, k0:k1, :]
                    ).then_inc(dma_x, 16)
                gp.wait_ge(s_snap, 16)
                gp.dma_start(
                    out=out[:, 16 * K:32 * K], in_=EXPB[:, 16 * K:32 * K]
                ).then_inc(dma_m, 16)
                gp.wait_ge(s_fin, 1)
                gp.dma_start(
                    out=out[:, 0:16 * K], in_=EXPB[:, 0:16 * K]
                ).then_inc(dma_m, 16)
                gp.dma_start(
                    out=out[:, 32 * K:32 * K + 16],
                    in_=EXPB[:, 32 * K:32 * K + 16],
                ).then_inc(dma_m, 16)

            @block.tensor
            def _(pe):
                pe.ldweights(WC[:, 0:L])._wait_ge(dma_m, 16)
                for k in range(D):
                    src = None if k == 0 else SIG[k % 2]
                    for G in range(NG):
                        rhs = (WC[:, L + GW * G:L + GW * (G + 1)]
                               if k == 0 else src[:, GW * G:GW * (G + 1)])
                        ins = pe.matmul(
                            PS[G][:, :],
                            lhsT=WC[:, 0:L],
                            rhs=rhs,
                            start=True,
                            stop=True,
                        )
                        ins.ins.ldweights = False
                        if k > 0:
                            ins._wait_ge(s_dv[G], k)
                        elif G == 0:
                            ins._wait_ge(dma_m, 32)
                        ins.then_inc(s_pe[G], 1)

            @block.vector
            def _(dv):
                ndum = 0
                for k in range(D):
                    if k == 0:
                        dv.tensor_copy(
                            DUM[:, 0:1], XG[0:1, 0, 0:1]
                        )._wait_ge(s_x0, 16)
                        ndum += 1
                    elif k in chunk_start:
                        dv.tensor_copy(
                            DUM[:, ndum % 16:ndum % 16 + 1], XG[0:1, 0, 0:1]
                        )._wait_ge(dma_x, 16 * (chunk_start[k] + 1))
                        ndum += 1
                    dst = SIG[(k + 1) % 2]
                    for G in range(NG):
                        c0 = GW * G
                        dv.tensor_mul(
                            dst[:, c0:c0 + GW],
                            PS[G][:, :],
                            XG[:, k, c0:c0 + GW],
                        )._wait_ge(s_pe[G], k + 1).then_inc(s_dv[G], 1)
                    if k == snap_step[1]:  # all snaps aligned at W-1
                        for G in range(NG):
                            ins = dv.tensor_copy(
                                EXPB[:, 16 * K + GW * G:16 * K + GW * (G + 1)],
                                dst[:, GW * G:GW * (G + 1)],
                            )
                            if G == NG - 1:
                                ins.then_inc(s_snap, 16)
                    if k == term_step[0]:  # chain 0's early terminal
                        dv.tensor_copy(
                            EXPB[:, 32 * K:32 * K + 16], dst[:, 0:16]
                        )
                    if k == D - 1:  # all other terminals
                        for G in range(NG):
                            dv.tensor_copy(
                                EXPB[:, GW * G:GW * (G + 1)],
                                dst[:, GW * G:GW * (G + 1)],
                            )
                dv.tensor_copy(DUM[:, 0:1], EXPB[0:1, 0:1]).then_inc(s_fin, 1)

    return nc


def _run_cores(nc, in_maps):
    from concourse.bass_utils import run_bass_kernel_spmd

    return run_bass_kernel_spmd(nc, in_maps, list(range(len(in_maps)))).results


def make_in_maps(inputs):
    """Host prep: lse-shift, exp, per-chain gather into step-major streams."""
    import ml_dtypes

    bf16 = ml_dtypes.bfloat16
    x = np.ascontiguousarray(np.asarray(inputs, dtype=np.float32))
    tr = _PROGRAM_CACHE["tr"]
    K, W = K_CHAINS, W_WARM
    D, starts, term_step, snap_step = plan(K, W)
    T = x.shape[1]

    xm = x.max(axis=2, keepdims=True)
    c = (np.log(np.sum(np.exp(x - xm), axis=2, keepdims=True)) + xm).astype(np.float32)
    ex = np.exp(x - c).astype(bf16)  # [B,T,L]

    E = np.exp(tr.astype(np.float32)).astype(bf16)

    posmat = np.empty((K, D), dtype=np.int64)
    for ci in range(K):
        posmat[ci] = np.arange(starts[ci], starts[ci] + D)
    posmat = np.minimum(posmat, T)

    in_maps = []
    for core in range(N_CORES):
        exc = ex[core * BL:(core + 1) * BL]  # [16, T, L]
        pad = np.concatenate(
            [exc, np.ones((BL, 1, L), dtype=bf16)], axis=1
        )  # [16, T+1, L]
        gat = pad[:, posmat, :]  # [16, K, D, L]
        xg = np.ascontiguousarray(
            np.transpose(gat, (3, 2, 1, 0)).reshape(L, D, 16 * K)
        )
        wc = np.ones((L, L + 16 * K), dtype=bf16)
        wc[:, 0:L] = E
        wc[:, L:L + 16] = exc[:, 0, :].T  # chain 0 seeded with e_0
        in_maps.append({"xg": xg, "wconst": np.ascontiguousarray(wc)})
    return in_maps, c


def finish(res, inputs, labels_idx, trans, c):
    """Combine exported chain states host-side in float64."""
    x = np.asarray(inputs)
    lab = np.asarray(labels_idx)
    tr = np.asarray(trans)
    K = K_CHAINS

    lnz = np.empty(B)
    for core in range(N_CORES):
        expb = np.asarray(res[core]["out"]).astype(np.float64)
        terms = [expb[:, 16 * ci:16 * ci + 16] for ci in range(K)]
        terms[0] = expb[:, 32 * K:32 * K + 16]  # chain 0's early terminal
        snaps = {ci: expb[:, 16 * (K + ci):16 * (K + ci) + 16]
                 for ci in range(1, K)}
        v = np.log(terms[K - 1].sum(axis=0))
        for ci in range(K - 1):
            v += np.log(terms[ci].sum(axis=0)) - np.log(snaps[ci + 1].sum(axis=0))
        lnz[core * BL:(core + 1) * BL] = v

    log_norm = lnz + c.astype(np.float64).sum(axis=1)[:, 0]
    lab64 = lab.astype(np.int64)
    xg = np.take_along_axis(x, lab64[..., None], axis=2)[..., 0].astype(np.float64)
    point = xg.sum(axis=1)
    trans_sc = tr[lab64[:, :-1], lab64[:, 1:]].astype(np.float64).sum(axis=1)
    return (log_norm - point - trans_sc)[:, None].astype(np.float32)


def kernel(inputs, labels_idx, trans):
    if "nc" not in _PROGRAM_CACHE:
        _PROGRAM_CACHE["nc"] = _build_program()
    _PROGRAM_CACHE["tr"] = np.ascontiguousarray(np.asarray(trans, dtype=np.float32))
    nc = _PROGRAM_CACHE["nc"]

    in_maps, c = make_in_maps(inputs)
    res = _run_cores(nc, in_maps)
    return finish(res, inputs, labels_idx, trans, c)


# revision 8
# speedup vs baseline: 1.0345x; 1.0345x over previous
"""Trainium2 Bass kernel for CRF NLL loss (nn_CRF_71571335021248).

Strategy
--------
Data-parallel over batch B=128 across 8 cores (16 sequences per core).

The forward-algorithm logsumexp scan is reformulated in exp space:
    sigma_t = (E^T sigma_{t-1}) * e_t          E = exp(trans), e_t = exp(x_t)
Host-side we subtract the per-(b,t) logsumexp of the emissions (the NLL is
invariant), so fp32/bf16 never overflow and no renormalization is needed.

Latency wall of the sequential scan is broken with K parallel chains per
core exploiting Perron-Frobenius contraction: the per-step map
x -> (E^T x) * e_t is a positive linear map whose matrix products contract
direction exponentially (diagonal scalings are Hilbert-metric isometries;
tau = tanh(Delta(E)/4) ~ 0.27 here). A chain seeded with ones at position
p - W has the exact direction at p up to tau^W (~1e-7 for W=12). Each chain
covers one T-segment; relative scales are recovered host-side from component
ratios at the segment overlap points; everything combines in float64 on the
host from tiny [96,16] exported state vectors.

All chains run FORWARD, so the single stationary E is loaded once and there
are ZERO ldweights in steady state. Chains are ganged: ONE matmul advances a
whole gang (moving operand [96, 16*g]) and ONE DVE tensor_mul evacuates the
gang's PSUM block (amortizing the ~125 ns DVE PSUM-access init), multiplying
by emission columns the host pre-gathered into a step-major stream tensor
[96, D, 16*K] (exp'd, bf16). Sequential depth drops from 511 (bidirectional
baseline) to D ~ 55 steps.

Chain boundaries are aligned so ALL warmup snapshots land on step W-1 and
all terminals on step D-1 (slack absorbed by chain 0, whose terminal is the
only straggler): exports are 2 gang-wide [96,16g] copies per gang plus one
[96,16] copy, instead of per-chain copies that would serialize the DVE.

The gold-path score (point + transition gathers) is computed host-side.
"""

import math

import numpy as np

B, L = 128, 96
T_FULL = 1024
N_CORES = 8
BL = B // N_CORES  # 16 sequences per core

# chain config
K_CHAINS = 30
N_GANGS = 3
W_WARM = 10
CHUNK0 = 2  # steps in first DMA chunk (fast start)
CHUNK_SZ = 8

_PROGRAM_CACHE: dict = {}


def plan(K=K_CHAINS, W=W_WARM, T=T_FULL):
    """Aligned chain layout: snapshots all at step W-1, terminals at D-1.

    chain 0: seeded with e_0, absorbs positions 1..D, useful span [0, b1-1],
             terminal at position b1-1 = step b1-2.
    chain c>=1: seeded with ones at position b1+(c-1)(D-W)-W, useful span of
             D-W positions, snapshot at step W-1, terminal at step D-1.
    """
    D = math.ceil((T - 1 + (K - 1) * W) / K)
    b1 = T - (K - 1) * (D - W)
    assert W + 1 <= b1 <= D + 1, (K, W, D, b1)
    starts = [1] + [b1 + (c - 1) * (D - W) - W for c in range(1, K)]
    term_step = [b1 - 2] + [D - 1] * (K - 1)
    snap_step = [None] + [W - 1] * (K - 1)
    for c in range(1, K):
        assert starts[c - 1] + term_step[c - 1] == starts[c] + snap_step[c]
    assert starts[K - 1] + D - 1 == T - 1
    return D, starts, term_step, snap_step


def _chunks(D):
    """gp-queue DMA chunks (graduated sizes); steps [0,2) go via the SP queue."""
    out = []
    k = 2
    for sz in (4, 6):
        out.append((k, min(k + sz, D)))
        k += sz
    while k < D:
        out.append((k, min(k + CHUNK_SZ, D)))
        k += CHUNK_SZ
    return out


def _build_program(K=K_CHAINS, NG=N_GANGS, W=W_WARM):
    from contextlib import ExitStack

    import concourse.bass as bass
    from concourse import mybir

    f32 = mybir.dt.float32
    bf16 = mybir.dt.bfloat16
    assert K % NG == 0
    g = K // NG
    GW = 16 * g  # gang width in columns
    D, starts, term_step, snap_step = plan(K, W)
    chunks = _chunks(D)
    # export layout: [terms 16K | snaps 16K | chain0 term 16]
    NEXP = 2 * K + 1

    nc = bass.Bass()
    xg = nc.dram_tensor("xg", [L, D, 16 * K], bf16, kind="ExternalInput")
    wconst = nc.dram_tensor("wconst", [L, L + 16 * K], bf16, kind="ExternalInput")
    out = nc.dram_tensor("out", [L, 16 * NEXP], bf16, kind="ExternalOutput")

    es = ExitStack()
    with es:
        sem = lambda name: es.enter_context(nc.semaphore(name))
        sbuf = lambda name, shape, dt: es.enter_context(nc.sbuf_tensor(name, shape, dt))
        psum = lambda name, shape: es.enter_context(nc.psum_tensor(name, shape, f32))

        dma_m = sem("dma_m")
        dma_x = sem("dma_x")
        s_x0 = sem("s_x0")
        s_snap = sem("s_snap")
        s_pe = [sem(f"s_pe{G}") for G in range(NG)]
        s_dv = [sem(f"s_dv{G}") for G in range(NG)]
        s_fin = sem("s_fin")

        WC = sbuf("WC", [L, L + 16 * K], bf16)
        XG = sbuf("XG", [L, D, 16 * K], bf16)
        SIG = [sbuf("SIG0", [L, 16 * K], bf16), sbuf("SIG1", [L, 16 * K], bf16)]
        EXPB = sbuf("EXPB", [L, 16 * NEXP], bf16)
        DUM = sbuf("DUM", [1, 16], bf16)

        PS = [psum(f"PS{G}", [L, GW]) for G in range(NG)]

        chunk_start = {k0: ci for ci, (k0, k1) in enumerate(chunks)}

        with nc.Block() as block:

            @block.sync
            def _(sp):
                sp.dma_start(out=WC[:], in_=wconst[:, :]).then_inc(dma_m, 16)
                sp.dma_start(out=XG[:, 0:2, :], in_=xg[:, 0:2, :]).then_inc(s_x0, 16)
                sp.wait_ge(dma_m, 64)

            @block.gpsimd
            def _(gp):
                gp.wait_ge(s_x0, 16)
                for (k0, k1) in chunks:
                    gp.dma_start(
                        out=XG[:, k0:k1, :], in_=xg[:, k0:k1, :]
                    ).then_inc(dma_x, 16)
                gp.wait_ge(s_snap, 16)
                gp.dma_start(
                    out=out[:, 16 * K:32 * K], in_=EXPB[:, 16 * K:32 * K]
                ).then_inc(dma_m, 16)
                gp.wait_ge(s_fin, 1)
                gp.dma_start(
                    out=out[:, 0:16 * K], in_=EXPB[:, 0:16 * K]
                ).then_inc(dma_m, 16)
                gp.dma_start(
                    out=out[:, 32 * K:32 * K + 16],
                    in_=EXPB[:, 32 * K:32 * K + 16],
                ).then_inc(dma_m, 16)

            @block.tensor
            def _(pe):
                pe.ldweights(WC[:, 0:L])._wait_ge(dma_m, 16)
                for k in range(D):
                    src = None if k == 0 else SIG[k % 2]
                    for G in range(NG):
                        rhs = (WC[:, L + GW * G:L + GW * (G + 1)]
                               if k == 0 else src[:, GW * G:GW * (G + 1)])
                        ins = pe.matmul(
                            PS[G][:, :],
                            lhsT=WC[:, 0:L],
                            rhs=rhs,
                            start=True,
                            stop=True,
                        )
                        ins.ins.ldweights = False
                        if k > 0:
                            ins._wait_ge(s_dv[G], k)
                        elif G == 0:
                            ins._wait_ge(dma_m, 16)
                        ins.then_inc(s_pe[G], 1)

            @block.vector
            def _(dv):
                ndum = 0
                for k in range(D):
                    if k == 0:
                        dv.tensor_copy(
                            DUM[:, 0:1], XG[0:1, 0, 0:1]
                        )._wait_ge(s_x0, 16)
                        ndum += 1
                    elif k in chunk_start:
                        dv.tensor_copy(
                            DUM[:, ndum % 16:ndum % 16 + 1], XG[0:1, 0, 0:1]
                        )._wait_ge(dma_x, 16 * (chunk_start[k] + 1))
                        ndum += 1
                    dst = SIG[(k + 1) % 2]
                    for G in range(NG):
                        c0 = GW * G
                        dv.tensor_mul(
                            dst[:, c0:c0 + GW],
                            PS[G][:, :],
                            XG[:, k, c0:c0 + GW],
                        )._wait_ge(s_pe[G], k + 1).then_inc(s_dv[G], 1)
                    if k == snap_step[1]:  # all snaps aligned at W-1
                        for G in range(NG):
                            ins = dv.tensor_copy(
                                EXPB[:, 16 * K + GW * G:16 * K + GW * (G + 1)],
                                dst[:, GW * G:GW * (G + 1)],
                            )
                            if G == NG - 1:
                                ins.then_inc(s_snap, 16)
                    if k == term_step[0]:  # chain 0's early terminal
                        dv.tensor_copy(
                            EXPB[:, 32 * K:32 * K + 16], dst[:, 0:16]
                        )
                    if k == D - 1:  # all other terminals
                        for G in range(NG):
                            dv.tensor_copy(
                                EXPB[:, GW * G:GW * (G + 1)],
                                dst[:, GW * G:GW * (G + 1)],
                            )
                dv.tensor_copy(DUM[:, 0:1], EXPB[0:1, 0:1]).then_inc(s_fin, 1)

    return nc


def _run_cores(nc, in_maps):
    from concourse.bass_utils import run_bass_kernel_spmd

    return run_bass_kernel_spmd(nc, in_maps, list(range(len(in_maps)))).results


def make_in_maps(inputs):
    """Host prep: lse-shift, exp, per-chain gather into step-major streams."""
    import ml_dtypes

    bf16 = ml_dtypes.bfloat16
    x = np.ascontiguousarray(np.asarray(inputs, dtype=np.float32))
    tr = _PROGRAM_CACHE["tr"]
    K, W = K_CHAINS, W_WARM
    D, starts, term_step, snap_step = plan(K, W)
    T = x.shape[1]

    xm = x.max(axis=2, keepdims=True)
    c = (np.log(np.sum(np.exp(x - xm), axis=2, keepdims=True)) + xm).astype(np.float32)
    ex = np.exp(x - c).astype(bf16)  # [B,T,L]

    E = np.exp(tr.astype(np.float32)).astype(bf16)

    posmat = np.empty((K, D), dtype=np.int64)
    for ci in range(K):
        posmat[ci] = np.arange(starts[ci], starts[ci] + D)
    posmat = np.minimum(posmat, T)

    in_maps = []
    for core in range(N_CORES):
        exc = ex[core * BL:(core + 1) * BL]  # [16, T, L]
        pad = np.concatenate(
            [exc, np.ones((BL, 1, L), dtype=bf16)], axis=1
        )  # [16, T+1, L]
        gat = pad[:, posmat, :]  # [16, K, D, L]
        xg = np.ascontiguousarray(
            np.transpose(gat, (3, 2, 1, 0)).reshape(L, D, 16 * K)
        )
        wc = np.ones((L, L + 16 * K), dtype=bf16)
        wc[:, 0:L] = E
        wc[:, L:L + 16] = exc[:, 0, :].T  # chain 0 seeded with e_0
        in_maps.append({"xg": xg, "wconst": np.ascontiguousarray(wc)})
    return in_maps, c


def finish(res, inputs, labels_idx, trans, c):
    """Combine exported chain states host-side in float64."""
    x = np.asarray(inputs)
    lab = np.asarray(labels_idx)
    tr = np.asarray(trans)
    K = K_CHAINS

    lnz = np.empty(B)
    for core in range(N_CORES):
        expb = np.asarray(res[core]["out"]).astype(np.float64)
        terms = [expb[:, 16 * ci:16 * ci + 16] for ci in range(K)]
        terms[0] = expb[:, 32 * K:32 * K + 16]  # chain 0's early terminal
        snaps = {ci: expb[:, 16 * (K + ci):16 * (K + ci) + 16]
                 for ci in range(1, K)}
        v = np.log(terms[K - 1].sum(axis=0))
        for ci in range(K - 1):
            v += np.log(terms[ci].sum(axis=0)) - np.log(snaps[ci + 1].sum(axis=0))
        lnz[core * BL:(core + 1) * BL] = v

    log_norm = lnz + c.astype(np.float64).sum(axis=1)[:, 0]
    lab64 = lab.astype(np.int64)
    xg = np.take_along_axis(x, lab64[..., None], axis=2)[..., 0].astype(np.float64)
    point = xg.sum(axis=1)
    trans_sc = tr[lab64[:, :-1], lab64[:, 1:]].astype(np.float64).sum(axis=1)
    return (log_norm - point - trans_sc)[:, None].astype(np.float32)


def kernel(inputs, labels_idx, trans):
    if "nc" not in _PROGRAM_CACHE:
        _PROGRAM_CACHE["nc"] = _build_program()
    _PROGRAM_CACHE["tr"] = np.ascontiguousarray(np.asarray(trans, dtype=np.float32))
    nc = _PROGRAM_CACHE["nc"]

    in_maps, c = make_in_maps(inputs)
    res = _run_cores(nc, in_maps)
    return finish(res, inputs, labels_idx, trans, c)


# revision 9
# speedup vs baseline: 1.0811x; 1.0451x over previous
"""Trainium2 Bass kernel for CRF NLL loss (nn_CRF_71571335021248).

Strategy
--------
Data-parallel over batch B=128 across 8 cores (16 sequences per core).

The forward-algorithm logsumexp scan is reformulated in exp space:
    sigma_t = (E^T sigma_{t-1}) * e_t          E = exp(trans), e_t = exp(x_t)
Host-side we subtract the per-(b,t) logsumexp of the emissions (the NLL is
invariant), so fp32/bf16 never overflow and no renormalization is needed.

Latency wall of the sequential scan is broken with K parallel chains per
core exploiting Perron-Frobenius contraction: the per-step map
x -> (E^T x) * e_t is a positive linear map whose matrix products contract
direction exponentially (diagonal scalings are Hilbert-metric isometries;
tau = tanh(Delta(E)/4) ~ 0.27 here). A chain seeded with ones at position
p - W has the exact direction at p up to tau^W (~1e-7 for W=12). Each chain
covers one T-segment; relative scales are recovered host-side from component
ratios at the segment overlap points; everything combines in float64 on the
host from tiny [96,16] exported state vectors.

All chains run FORWARD, so the single stationary E is loaded once and there
are ZERO ldweights in steady state. Chains are ganged: ONE matmul advances a
whole gang (moving operand [96, 16*g]) and ONE DVE tensor_mul evacuates the
gang's PSUM block (amortizing the ~125 ns DVE PSUM-access init), multiplying
by emission columns the host pre-gathered into a step-major stream tensor
[96, D, 16*K] (exp'd, bf16). Sequential depth drops from 511 (bidirectional
baseline) to D ~ 55 steps.

Chain boundaries are aligned so ALL warmup snapshots land on step W-1 and
all terminals on step D-1 (slack absorbed by chain 0, whose terminal is the
only straggler): exports are 2 gang-wide [96,16g] copies per gang plus one
[96,16] copy, instead of per-chain copies that would serialize the DVE.

The gold-path score (point + transition gathers) is computed host-side.
"""

import math

import numpy as np

B, L = 128, 96
T_FULL = 1024
N_CORES = 8
BL = B // N_CORES  # 16 sequences per core

# chain config
K_CHAINS = 30
N_GANGS = 3
W_WARM = 10
CHUNK0 = 2  # steps in first DMA chunk (fast start)
CHUNK_SZ = 8

_PROGRAM_CACHE: dict = {}


def plan(K=K_CHAINS, W=W_WARM, T=T_FULL):
    """Aligned chain layout: snapshots all at step W-1, terminals at D-1.

    chain 0: seeded with e_0, absorbs positions 1..D, useful span [0, b1-1],
             terminal at position b1-1 = step b1-2.
    chain c>=1: seeded with ones at position b1+(c-1)(D-W)-W, useful span of
             D-W positions, snapshot at step W-1, terminal at step D-1.
    """
    D = math.ceil((T - 1 + (K - 1) * W) / K)
    b1 = T - (K - 1) * (D - W)
    assert W + 1 <= b1 <= D + 1, (K, W, D, b1)
    starts = [1] + [b1 + (c - 1) * (D - W) - W for c in range(1, K)]
    term_step = [b1 - 2] + [D - 1] * (K - 1)
    snap_step = [None] + [W - 1] * (K - 1)
    for c in range(1, K):
        assert starts[c - 1] + term_step[c - 1] == starts[c] + snap_step[c]
    assert starts[K - 1] + D - 1 == T - 1
    return D, starts, term_step, snap_step


def _chunks(D):
    """gp-queue DMA chunks; steps [0,6) go via the SP queue in two chunks."""
    out = [(6, 12)]
    k = 12
    while k < D:
        out.append((k, min(k + 16, D)))
        k += 16
    return out


def _build_program(K=K_CHAINS, NG=N_GANGS, W=W_WARM):
    from contextlib import ExitStack

    import concourse.bass as bass
    from concourse import mybir

    f32 = mybir.dt.float32
    bf16 = mybir.dt.bfloat16
    assert K % NG == 0
    g = K // NG
    GW = 16 * g  # gang width in columns
    D, starts, term_step, snap_step = plan(K, W)
    chunks = _chunks(D)
    # export layout: [terms 16K | c0 term 16 | scratch GW-16 | snaps 16K]
    off_c0 = 16 * K
    off_snap = 16 * K + GW
    NEXP = 2 * K + g

    nc = bass.Bass()
    xg = nc.dram_tensor("xg", [L, D, 16 * K], bf16, kind="ExternalInput")
    wconst = nc.dram_tensor("wconst", [L, L + 16 * K], bf16, kind="ExternalInput")
    out = nc.dram_tensor("out", [L, 16 * NEXP], bf16, kind="ExternalOutput")

    es = ExitStack()
    with es:
        sem = lambda name: es.enter_context(nc.semaphore(name))
        sbuf = lambda name, shape, dt: es.enter_context(nc.sbuf_tensor(name, shape, dt))
        psum = lambda name, shape: es.enter_context(nc.psum_tensor(name, shape, f32))

        dma_m = sem("dma_m")
        dma_x = sem("dma_x")
        s_x0 = sem("s_x0")
        s_pe = [sem(f"s_pe{G}") for G in range(NG)]
        s_dv = [sem(f"s_dv{G}") for G in range(NG)]

        WC = sbuf("WC", [L, L + 16 * K], bf16)
        XG = sbuf("XG", [L, D, 16 * K], bf16)
        SIG = [sbuf("SIG0", [L, 16 * K], bf16), sbuf("SIG1", [L, 16 * K], bf16)]
        EXPB = sbuf("EXPB", [L, 16 * NEXP], bf16)
        DUM = sbuf("DUM", [1, 16], bf16)

        PS = [psum(f"PS{G}", [L, GW]) for G in range(NG)]

        chunk_start = {k0: ci for ci, (k0, k1) in enumerate(chunks)}

        with nc.Block() as block:

            @block.sync
            def _(sp):
                sp.dma_start(out=WC[:], in_=wconst[:, :]).then_inc(dma_m, 16)
                sp.dma_start(out=XG[:, 0:2, :], in_=xg[:, 0:2, :]).then_inc(s_x0, 16)
                sp.dma_start(out=XG[:, 2:6, :], in_=xg[:, 2:6, :]).then_inc(s_x0, 16)
                sp.wait_ge(dma_m, 48)

            @block.gpsimd
            def _(gp):
                gp.wait_ge(dma_m, 16)
                for (k0, k1) in chunks:
                    gp.dma_start(
                        out=XG[:, k0:k1, :], in_=xg[:, k0:k1, :]
                    ).then_inc(dma_x, 16)
                gp.wait_ge(s_dv[NG - 1], W)
                gp.dma_start(
                    out=out[:, off_snap:off_snap + 16 * K],
                    in_=EXPB[:, off_snap:off_snap + 16 * K],
                ).then_inc(dma_m, 16)
                gp.wait_ge(s_dv[NG - 1], D)
                gp.dma_start(
                    out=out[:, 0:16 * K + 16], in_=EXPB[:, 0:16 * K + 16]
                ).then_inc(dma_m, 16)

            def state_ap(k, G):
                """Where the gang-G state after step k lives (TT_k's dst)."""
                if k == snap_step[1]:
                    return EXPB[:, off_snap + GW * G:off_snap + GW * (G + 1)]
                if k == term_step[0] and G == 0:
                    return EXPB[:, off_c0:off_c0 + GW]
                if k == D - 1:
                    return EXPB[:, GW * G:GW * (G + 1)]
                return SIG[(k + 1) % 2][:, GW * G:GW * (G + 1)]

            @block.tensor
            def _(pe):
                pe.ldweights(WC[:, 0:L])._wait_ge(dma_m, 16)
                for k in range(D):
                    for G in range(NG):
                        rhs = (WC[:, L + GW * G:L + GW * (G + 1)]
                               if k == 0 else state_ap(k - 1, G))
                        ins = pe.matmul(
                            PS[G][:, :],
                            lhsT=WC[:, 0:L],
                            rhs=rhs,
                            start=True,
                            stop=True,
                        )
                        ins.ins.ldweights = False
                        if k > 0:
                            ins._wait_ge(s_dv[G], k)
                        ins.then_inc(s_pe[G], 1)

            @block.vector
            def _(dv):
                ndum = 0
                sp_chunk = {0: 16, 2: 32}
                gp_chunk = {k0: 16 * (ci + 1) for ci, (k0, k1) in enumerate(chunks)}
                for k in range(D):
                    if k in sp_chunk:
                        dv.tensor_copy(
                            DUM[:, ndum % 16:ndum % 16 + 1], XG[0:1, 0, 0:1]
                        )._wait_ge(s_x0, sp_chunk[k])
                        ndum += 1
                    elif k in gp_chunk:
                        dv.tensor_copy(
                            DUM[:, ndum % 16:ndum % 16 + 1], XG[0:1, 0, 0:1]
                        )._wait_ge(dma_x, gp_chunk[k])
                        ndum += 1
                    for G in range(NG):
                        dv.tensor_mul(
                            state_ap(k, G),
                            PS[G][:, :],
                            XG[:, k, GW * G:GW * (G + 1)],
                        )._wait_ge(s_pe[G], k + 1).then_inc(s_dv[G], 1)

    return nc


def _run_cores(nc, in_maps):
    from concourse.bass_utils import run_bass_kernel_spmd

    return run_bass_kernel_spmd(nc, in_maps, list(range(len(in_maps)))).results


def make_in_maps(inputs):
    """Host prep: lse-shift, exp, per-chain gather into step-major streams."""
    import ml_dtypes

    bf16 = ml_dtypes.bfloat16
    x = np.ascontiguousarray(np.asarray(inputs, dtype=np.float32))
    tr = _PROGRAM_CACHE["tr"]
    K, W = K_CHAINS, W_WARM
    D, starts, term_step, snap_step = plan(K, W)
    T = x.shape[1]

    xm = x.max(axis=2, keepdims=True)
    c = (np.log(np.sum(np.exp(x - xm), axis=2, keepdims=True)) + xm).astype(np.float32)
    ex = np.exp(x - c).astype(bf16)  # [B,T,L]

    E = np.exp(tr.astype(np.float32)).astype(bf16)

    posmat = np.empty((K, D), dtype=np.int64)
    for ci in range(K):
        posmat[ci] = np.arange(starts[ci], starts[ci] + D)
    posmat = np.minimum(posmat, T)

    in_maps = []
    for core in range(N_CORES):
        exc = ex[core * BL:(core + 1) * BL]  # [16, T, L]
        pad = np.concatenate(
            [exc, np.ones((BL, 1, L), dtype=bf16)], axis=1
        )  # [16, T+1, L]
        gat = pad[:, posmat, :]  # [16, K, D, L]
        xg = np.ascontiguousarray(
            np.transpose(gat, (3, 2, 1, 0)).reshape(L, D, 16 * K)
        )
        wc = np.ones((L, L + 16 * K), dtype=bf16)
        wc[:, 0:L] = E
        wc[:, L:L + 16] = exc[:, 0, :].T  # chain 0 seeded with e_0
        in_maps.append({"xg": xg, "wconst": np.ascontiguousarray(wc)})
    return in_maps, c


def finish(res, inputs, labels_idx, trans, c):
    """Combine exported chain states host-side in float64."""
    x = np.asarray(inputs)
    lab = np.asarray(labels_idx)
    tr = np.asarray(trans)
    K = K_CHAINS

    lnz = np.empty(B)
    for core in range(N_CORES):
        expb = np.asarray(res[core]["out"]).astype(np.float64)
        g = K // N_GANGS
        off_snap = 16 * K + 16 * g
        terms = [expb[:, 16 * ci:16 * ci + 16] for ci in range(K)]
        terms[0] = expb[:, 16 * K:16 * K + 16]  # chain 0's early terminal
        snaps = {ci: expb[:, off_snap + 16 * ci:off_snap + 16 * ci + 16]
                 for ci in range(1, K)}
        v = np.log(terms[K - 1].sum(axis=0))
        for ci in range(K - 1):
            v += np.log(terms[ci].sum(axis=0)) - np.log(snaps[ci + 1].sum(axis=0))
        lnz[core * BL:(core + 1) * BL] = v

    log_norm = lnz + c.astype(np.float64).sum(axis=1)[:, 0]
    lab64 = lab.astype(np.int64)
    xg = np.take_along_axis(x, lab64[..., None], axis=2)[..., 0].astype(np.float64)
    point = xg.sum(axis=1)
    trans_sc = tr[lab64[:, :-1], lab64[:, 1:]].astype(np.float64).sum(axis=1)
    return (log_norm - point - trans_sc)[:, None].astype(np.float32)


def kernel(inputs, labels_idx, trans):
    if "nc" not in _PROGRAM_CACHE:
        _PROGRAM_CACHE["nc"] = _build_program()
    _PROGRAM_CACHE["tr"] = np.ascontiguousarray(np.asarray(trans, dtype=np.float32))
    nc = _PROGRAM_CACHE["nc"]

    in_maps, c = make_in_maps(inputs)
    res = _run_cores(nc, in_maps)
    return finish(res, inputs, labels_idx, trans, c)


# revision 10
# speedup vs baseline: 1.1499x; 1.0636x over previous
"""Trainium2 Bass kernel for CRF NLL loss (nn_CRF_71571335021248).

Strategy
--------
Data-parallel over batch B=128 across 8 cores (16 sequences per core).

The forward-algorithm logsumexp scan is reformulated in exp space:
    sigma_t = (E^T sigma_{t-1}) * e_t          E = exp(trans), e_t = exp(x_t)
Host-side we subtract the per-(b,t) logsumexp of the emissions (the NLL is
invariant), so fp32/bf16 never overflow and no renormalization is needed.

Latency wall of the sequential scan is broken with K parallel chains per
core exploiting Perron-Frobenius contraction: the per-step map
x -> (E^T x) * e_t is a positive linear map whose matrix products contract
direction exponentially (diagonal scalings are Hilbert-metric isometries;
tau = tanh(Delta(E)/4) ~ 0.27 here). A chain seeded with ones at position
p - W has the exact direction at p up to tau^W (~1e-7 for W=12). Each chain
covers one T-segment; relative scales are recovered host-side from component
ratios at the segment overlap points; everything combines in float64 on the
host from tiny [96,16] exported state vectors.

All chains run FORWARD, so the single stationary E is loaded once and there
are ZERO ldweights in steady state. Chains are ganged: ONE matmul advances a
whole gang (moving operand [96, 16*g]) and ONE DVE tensor_mul evacuates the
gang's PSUM block (amortizing the ~125 ns DVE PSUM-access init), multiplying
by emission columns the host pre-gathered into a step-major stream tensor
[96, D, 16*K] (exp'd, bf16). Sequential depth drops from 511 (bidirectional
baseline) to D ~ 55 steps.

Chain boundaries are aligned so ALL warmup snapshots land on step W-1 and
all terminals on step D-1 (slack absorbed by chain 0, whose terminal is the
only straggler): exports are 2 gang-wide [96,16g] copies per gang plus one
[96,16] copy, instead of per-chain copies that would serialize the DVE.

The gold-path score (point + transition gathers) is computed host-side.
"""

import math

import numpy as np

B, L = 128, 96
T_FULL = 1024
N_CORES = 8
BL = B // N_CORES  # 16 sequences per core

# chain config
K_CHAINS = 33
N_GANGS = 3
W_WARM = 10
CHUNK0 = 2  # steps in first DMA chunk (fast start)
CHUNK_SZ = 8

_PROGRAM_CACHE: dict = {}


def plan(K=K_CHAINS, W=W_WARM, T=T_FULL):
    """Aligned chain layout: snapshots all at step W-1, terminals at D-1.

    chain 0: seeded with e_0, absorbs positions 1..D, useful span [0, b1-1],
             terminal at position b1-1 = step b1-2.
    chain c>=1: seeded with ones at position b1+(c-1)(D-W)-W, useful span of
             D-W positions, snapshot at step W-1, terminal at step D-1.
    """
    D = math.ceil((T - 1 + (K - 1) * W) / K)
    b1 = T - (K - 1) * (D - W)
    assert W + 1 <= b1 <= D + 1, (K, W, D, b1)
    starts = [1] + [b1 + (c - 1) * (D - W) - W for c in range(1, K)]
    term_step = [b1 - 2] + [D - 1] * (K - 1)
    snap_step = [None] + [W - 1] * (K - 1)
    for c in range(1, K):
        assert starts[c - 1] + term_step[c - 1] == starts[c] + snap_step[c]
    assert starts[K - 1] + D - 1 == T - 1
    return D, starts, term_step, snap_step


def _chunks(D):
    """gp-queue DMA chunks; steps [0,6) go via the SP queue in two chunks."""
    out = []
    k = 6
    for sz in (4, 4, 4, 8, 8):
        out.append((k, min(k + sz, D)))
        k += sz
        if k >= D:
            return out
    while k < D:
        out.append((k, min(k + 10, D)))
        k += 10
    return out


def _build_program(K=K_CHAINS, NG=N_GANGS, W=W_WARM):
    from contextlib import ExitStack

    import concourse.bass as bass
    from concourse import mybir

    f32 = mybir.dt.float32
    bf16 = mybir.dt.bfloat16
    assert K % NG == 0
    g = K // NG
    GW = 16 * g  # gang width in columns
    D, starts, term_step, snap_step = plan(K, W)
    chunks = _chunks(D)
    # export layout: [terms 16K | c0 term 16 | scratch GW-16 | snaps 16K]
    off_c0 = 16 * K
    off_snap = 16 * K + GW
    NEXP = 2 * K + g

    nc = bass.Bass()
    xg = nc.dram_tensor("xg", [L, D, 16 * K], bf16, kind="ExternalInput")
    wconst = nc.dram_tensor("wconst", [L, L + 16 * K], bf16, kind="ExternalInput")
    out = nc.dram_tensor("out", [L, 16 * NEXP], bf16, kind="ExternalOutput")

    es = ExitStack()
    with es:
        sem = lambda name: es.enter_context(nc.semaphore(name))
        sbuf = lambda name, shape, dt: es.enter_context(nc.sbuf_tensor(name, shape, dt))
        psum = lambda name, shape: es.enter_context(nc.psum_tensor(name, shape, f32))

        dma_m = sem("dma_m")
        dma_x = sem("dma_x")
        s_x0 = sem("s_x0")
        s_pe = [sem(f"s_pe{G}") for G in range(NG)]
        s_dv = [sem(f"s_dv{G}") for G in range(NG)]

        WC = sbuf("WC", [L, L + 16 * K], bf16)
        XG = sbuf("XG", [L, D, 16 * K], bf16)
        SIG = [sbuf("SIG0", [L, 16 * K], bf16), sbuf("SIG1", [L, 16 * K], bf16)]
        EXPB = sbuf("EXPB", [L, 16 * NEXP], bf16)
        DUM = sbuf("DUM", [1, 16], bf16)

        PS = [psum(f"PS{G}", [L, GW]) for G in range(NG)]

        chunk_start = {k0: ci for ci, (k0, k1) in enumerate(chunks)}

        with nc.Block() as block:

            @block.sync
            def _(sp):
                sp.dma_start(out=WC[:], in_=wconst[:, :]).then_inc(dma_m, 16)
                sp.dma_start(out=XG[:, 0:2, :], in_=xg[:, 0:2, :]).then_inc(s_x0, 16)
                sp.dma_start(out=XG[:, 2:6, :], in_=xg[:, 2:6, :]).then_inc(s_x0, 16)
                sp.wait_ge(s_dv[NG - 1], D)
                sp.dma_start(
                    out=out[:, 0:16 * K + 16], in_=EXPB[:, 0:16 * K + 16]
                ).then_inc(dma_m, 16)
                sp.wait_ge(dma_m, 48)

            @block.gpsimd
            def _(gp):
                gp.wait_ge(dma_m, 16)
                for (k0, k1) in chunks:
                    gp.dma_start(
                        out=XG[:, k0:k1, :], in_=xg[:, k0:k1, :]
                    ).then_inc(dma_x, 16)
                gp.wait_ge(s_dv[NG - 1], W)
                gp.dma_start(
                    out=out[:, off_snap:off_snap + 16 * K],
                    in_=EXPB[:, off_snap:off_snap + 16 * K],
                ).then_inc(dma_m, 16)


            def state_ap(k, G):
                """Where the gang-G state after step k lives (TT_k's dst)."""
                if k == snap_step[1]:
                    return EXPB[:, off_snap + GW * G:off_snap + GW * (G + 1)]
                if k == term_step[0] and G == 0:
                    return EXPB[:, off_c0:off_c0 + GW]
                if k == D - 1:
                    return EXPB[:, GW * G:GW * (G + 1)]
                return SIG[(k + 1) % 2][:, GW * G:GW * (G + 1)]

            @block.tensor
            def _(pe):
                pe.ldweights(WC[:, 0:L])._wait_ge(dma_m, 16)
                for k in range(D):
                    for G in range(NG):
                        rhs = (WC[:, L + GW * G:L + GW * (G + 1)]
                               if k == 0 else state_ap(k - 1, G))
                        ins = pe.matmul(
                            PS[G][:, :],
                            lhsT=WC[:, 0:L],
                            rhs=rhs,
                            start=True,
                            stop=True,
                        )
                        ins.ins.ldweights = False
                        if k > 0:
                            ins._wait_ge(s_dv[G], k)
                        ins.then_inc(s_pe[G], 1)

            @block.vector
            def _(dv):
                ndum = 0
                sp_chunk = {0: 16, 2: 32}
                gp_chunk = {k0: 16 * (ci + 1) for ci, (k0, k1) in enumerate(chunks)}
                for k in range(D):
                    if k in sp_chunk:
                        dv.tensor_copy(
                            DUM[:, ndum % 16:ndum % 16 + 1], XG[0:1, 0, 0:1]
                        )._wait_ge(s_x0, sp_chunk[k])
                        ndum += 1
                    elif k in gp_chunk:
                        dv.tensor_copy(
                            DUM[:, ndum % 16:ndum % 16 + 1], XG[0:1, 0, 0:1]
                        )._wait_ge(dma_x, gp_chunk[k])
                        ndum += 1
                    for G in range(NG):
                        dv.tensor_mul(
                            state_ap(k, G),
                            PS[G][:, :],
                            XG[:, k, GW * G:GW * (G + 1)],
                        )._wait_ge(s_pe[G], k + 1).then_inc(s_dv[G], 1)

    return nc


def _run_cores(nc, in_maps):
    from concourse.bass_utils import run_bass_kernel_spmd

    return run_bass_kernel_spmd(nc, in_maps, list(range(len(in_maps)))).results


def make_in_maps(inputs):
    """Host prep: lse-shift, exp, per-chain gather into step-major streams."""
    import ml_dtypes

    bf16 = ml_dtypes.bfloat16
    x = np.ascontiguousarray(np.asarray(inputs, dtype=np.float32))
    tr = _PROGRAM_CACHE["tr"]
    K, W = K_CHAINS, W_WARM
    D, starts, term_step, snap_step = plan(K, W)
    T = x.shape[1]

    xm = x.max(axis=2, keepdims=True)
    c = (np.log(np.sum(np.exp(x - xm), axis=2, keepdims=True)) + xm).astype(np.float32)
    ex = np.exp(x - c).astype(bf16)  # [B,T,L]

    E = np.exp(tr.astype(np.float32)).astype(bf16)

    posmat = np.empty((K, D), dtype=np.int64)
    for ci in range(K):
        posmat[ci] = np.arange(starts[ci], starts[ci] + D)
    posmat = np.minimum(posmat, T)

    in_maps = []
    for core in range(N_CORES):
        exc = ex[core * BL:(core + 1) * BL]  # [16, T, L]
        pad = np.concatenate(
            [exc, np.ones((BL, 1, L), dtype=bf16)], axis=1
        )  # [16, T+1, L]
        gat = pad[:, posmat, :]  # [16, K, D, L]
        xg = np.ascontiguousarray(
            np.transpose(gat, (3, 2, 1, 0)).reshape(L, D, 16 * K)
        )
        wc = np.ones((L, L + 16 * K), dtype=bf16)
        wc[:, 0:L] = E
        wc[:, L:L + 16] = exc[:, 0, :].T  # chain 0 seeded with e_0
        in_maps.append({"xg": xg, "wconst": np.ascontiguousarray(wc)})
    return in_maps, c


def finish(res, inputs, labels_idx, trans, c):
    """Combine exported chain states host-side in float64."""
    x = np.asarray(inputs)
    lab = np.asarray(labels_idx)
    tr = np.asarray(trans)
    K = K_CHAINS

    lnz = np.empty(B)
    for core in range(N_CORES):
        expb = np.asarray(res[core]["out"]).astype(np.float64)
        g = K // N_GANGS
        off_snap = 16 * K + 16 * g
        terms = [expb[:, 16 * ci:16 * ci + 16] for ci in range(K)]
        terms[0] = expb[:, 16 * K:16 * K + 16]  # chain 0's early terminal
        snaps = {ci: expb[:, off_snap + 16 * ci:off_snap + 16 * ci + 16]
                 for ci in range(1, K)}
        v = np.log(terms[K - 1].sum(axis=0))
        for ci in range(K - 1):
            v += np.log(terms[ci].sum(axis=0)) - np.log(snaps[ci + 1].sum(axis=0))
        lnz[core * BL:(core + 1) * BL] = v

    log_norm = lnz + c.astype(np.float64).sum(axis=1)[:, 0]
    lab64 = lab.astype(np.int64)
    xg = np.take_along_axis(x, lab64[..., None], axis=2)[..., 0].astype(np.float64)
    point = xg.sum(axis=1)
    trans_sc = tr[lab64[:, :-1], lab64[:, 1:]].astype(np.float64).sum(axis=1)
    return (log_norm - point - trans_sc)[:, None].astype(np.float32)


def kernel(inputs, labels_idx, trans):
    if "nc" not in _PROGRAM_CACHE:
        _PROGRAM_CACHE["nc"] = _build_program()
    _PROGRAM_CACHE["tr"] = np.ascontiguousarray(np.asarray(trans, dtype=np.float32))
    nc = _PROGRAM_CACHE["nc"]

    in_maps, c = make_in_maps(inputs)
    res = _run_cores(nc, in_maps)
    return finish(res, inputs, labels_idx, trans, c)


# revision 11
# speedup vs baseline: 1.2513x; 1.0882x over previous
"""Trainium2 Bass kernel for CRF NLL loss (nn_CRF_71571335021248).

Strategy
--------
Data-parallel over batch B=128 across 8 cores (16 sequences per core).

The forward-algorithm logsumexp scan is reformulated in exp space:
    sigma_t = (E^T sigma_{t-1}) * e_t          E = exp(trans), e_t = exp(x_t)
Host-side we subtract the per-(b,t) logsumexp of the emissions (the NLL is
invariant), so fp32/bf16 never overflow and no renormalization is needed.

Latency wall of the sequential scan is broken with K parallel chains per
core exploiting Perron-Frobenius contraction: the per-step map
x -> (E^T x) * e_t is a positive linear map whose matrix products contract
direction exponentially (diagonal scalings are Hilbert-metric isometries;
tau = tanh(Delta(E)/4) ~ 0.27 here). A chain seeded with ones at position
p - W has the exact direction at p up to tau^W (~1e-7 for W=12). Each chain
covers one T-segment; relative scales are recovered host-side from component
ratios at the segment overlap points; everything combines in float64 on the
host from tiny [96,16] exported state vectors.

All chains run FORWARD, so the single stationary E is loaded once and there
are ZERO ldweights in steady state. Chains are ganged: ONE matmul advances a
whole gang (moving operand [96, 16*g]) and ONE DVE tensor_mul evacuates the
gang's PSUM block (amortizing the ~125 ns DVE PSUM-access init), multiplying
by emission columns the host pre-gathered into a step-major stream tensor
[96, D, 16*K] (exp'd, bf16). Sequential depth drops from 511 (bidirectional
baseline) to D ~ 55 steps.

Chain boundaries are aligned so ALL warmup snapshots land on step W-1 and
all terminals on step D-1 (slack absorbed by chain 0, whose terminal is the
only straggler): exports are 2 gang-wide [96,16g] copies per gang plus one
[96,16] copy, instead of per-chain copies that would serialize the DVE.

The gold-path score (point + transition gathers) is computed host-side.
"""

import math

import numpy as np

B, L = 128, 96
T_FULL = 1024
N_CORES = 8
BL = B // N_CORES  # 16 sequences per core

# chain config
K_CHAINS = 33
N_GANGS = 3
W_WARM = 4
CHUNK0 = 2  # steps in first DMA chunk (fast start)
CHUNK_SZ = 8

_PROGRAM_CACHE: dict = {}


def plan(K=K_CHAINS, W=W_WARM, T=T_FULL):
    """Aligned chain layout: snapshots all at step W-1, terminals at D-1.

    chain 0: seeded with e_0, absorbs positions 1..D, useful span [0, b1-1],
             terminal at position b1-1 = step b1-2.
    chain c>=1: seeded with ones at position b1+(c-1)(D-W)-W, useful span of
             D-W positions, snapshot at step W-1, terminal at step D-1.
    """
    D = math.ceil((T - 1 + (K - 1) * W) / K)
    b1 = T - (K - 1) * (D - W)
    assert W + 1 <= b1 <= D + 1, (K, W, D, b1)
    starts = [1] + [b1 + (c - 1) * (D - W) - W for c in range(1, K)]
    term_step = [b1 - 2] + [D - 1] * (K - 1)
    snap_step = [None] + [W - 1] * (K - 1)
    for c in range(1, K):
        assert starts[c - 1] + term_step[c - 1] == starts[c] + snap_step[c]
    assert starts[K - 1] + D - 1 == T - 1
    return D, starts, term_step, snap_step


def _chunks(D):
    """gp-queue DMA chunks; steps [0,6) go via the SP queue in two chunks."""
    out = []
    k = 6
    for sz in (4, 4, 4, 8, 8):
        out.append((k, min(k + sz, D)))
        k += sz
        if k >= D:
            return out
    while k < D:
        out.append((k, min(k + 10, D)))
        k += 10
    return out


def _build_program(K=K_CHAINS, NG=N_GANGS, W=W_WARM):
    from contextlib import ExitStack

    import concourse.bass as bass
    from concourse import mybir

    f32 = mybir.dt.float32
    bf16 = mybir.dt.bfloat16
    assert K % NG == 0
    g = K // NG
    GW = 16 * g  # gang width in columns
    D, starts, term_step, snap_step = plan(K, W)
    chunks = _chunks(D)
    # export layout: [terms 16K | c0 term 16 | scratch GW-16 | snaps 16K]
    off_c0 = 16 * K
    off_snap = 16 * K + GW
    NEXP = 2 * K + g

    nc = bass.Bass()
    xg = nc.dram_tensor("xg", [L, D, 16 * K], bf16, kind="ExternalInput")
    wconst = nc.dram_tensor("wconst", [L, L + 16 * K], bf16, kind="ExternalInput")
    out = nc.dram_tensor("out", [L, 16 * NEXP], bf16, kind="ExternalOutput")

    es = ExitStack()
    with es:
        sem = lambda name: es.enter_context(nc.semaphore(name))
        sbuf = lambda name, shape, dt: es.enter_context(nc.sbuf_tensor(name, shape, dt))
        psum = lambda name, shape: es.enter_context(nc.psum_tensor(name, shape, f32))

        dma_m = sem("dma_m")
        dma_x = sem("dma_x")
        s_x0 = sem("s_x0")
        s_pe = [sem(f"s_pe{G}") for G in range(NG)]
        s_dv = [sem(f"s_dv{G}") for G in range(NG)]

        WC = sbuf("WC", [L, L + 16 * K], bf16)
        XG = sbuf("XG", [L, D, 16 * K], bf16)
        SIG = [sbuf("SIG0", [L, 16 * K], bf16), sbuf("SIG1", [L, 16 * K], bf16)]
        EXPB = sbuf("EXPB", [L, 16 * NEXP], bf16)
        DUM = sbuf("DUM", [1, 16], bf16)

        PS = [psum(f"PS{G}", [L, GW]) for G in range(NG)]

        chunk_start = {k0: ci for ci, (k0, k1) in enumerate(chunks)}

        with nc.Block() as block:

            @block.sync
            def _(sp):
                sp.dma_start(out=WC[:], in_=wconst[:, :]).then_inc(dma_m, 16)
                sp.dma_start(out=XG[:, 0:2, :], in_=xg[:, 0:2, :]).then_inc(s_x0, 16)
                sp.dma_start(out=XG[:, 2:6, :], in_=xg[:, 2:6, :]).then_inc(s_x0, 16)
                sp.wait_ge(s_dv[NG - 1], D)
                sp.dma_start(
                    out=out[:, 0:16 * K + 16], in_=EXPB[:, 0:16 * K + 16]
                ).then_inc(dma_m, 16)
                sp.wait_ge(dma_m, 48)

            @block.gpsimd
            def _(gp):
                gp.wait_ge(dma_m, 16)
                for (k0, k1) in chunks:
                    gp.dma_start(
                        out=XG[:, k0:k1, :], in_=xg[:, k0:k1, :]
                    ).then_inc(dma_x, 16)
                gp.wait_ge(s_dv[NG - 1], W)
                gp.dma_start(
                    out=out[:, off_snap:off_snap + 16 * K],
                    in_=EXPB[:, off_snap:off_snap + 16 * K],
                ).then_inc(dma_m, 16)


            def state_ap(k, G):
                """Where the gang-G state after step k lives (TT_k's dst)."""
                if k == snap_step[1]:
                    return EXPB[:, off_snap + GW * G:off_snap + GW * (G + 1)]
                if k == term_step[0] and G == 0:
                    return EXPB[:, off_c0:off_c0 + GW]
                if k == D - 1:
                    return EXPB[:, GW * G:GW * (G + 1)]
                return SIG[(k + 1) % 2][:, GW * G:GW * (G + 1)]

            @block.tensor
            def _(pe):
                pe.ldweights(WC[:, 0:L])._wait_ge(dma_m, 16)
                for k in range(D):
                    for G in range(NG):
                        rhs = (WC[:, L + GW * G:L + GW * (G + 1)]
                               if k == 0 else state_ap(k - 1, G))
                        ins = pe.matmul(
                            PS[G][:, :],
                            lhsT=WC[:, 0:L],
                            rhs=rhs,
                            start=True,
                            stop=True,
                        )
                        ins.ins.ldweights = False
                        if k > 0:
                            ins._wait_ge(s_dv[G], k)
                        ins.then_inc(s_pe[G], 1)

            @block.vector
            def _(dv):
                ndum = 0
                sp_chunk = {0: 16, 2: 32}
                gp_chunk = {k0: 16 * (ci + 1) for ci, (k0, k1) in enumerate(chunks)}
                for k in range(D):
                    if k in sp_chunk:
                        dv.tensor_copy(
                            DUM[:, ndum % 16:ndum % 16 + 1], XG[0:1, 0, 0:1]
                        )._wait_ge(s_x0, sp_chunk[k])
                        ndum += 1
                    elif k in gp_chunk:
                        dv.tensor_copy(
                            DUM[:, ndum % 16:ndum % 16 + 1], XG[0:1, 0, 0:1]
                        )._wait_ge(dma_x, gp_chunk[k])
                        ndum += 1
                    for G in range(NG):
                        dv.tensor_mul(
                            state_ap(k, G),
                            PS[G][:, :],
                            XG[:, k, GW * G:GW * (G + 1)],
                        )._wait_ge(s_pe[G], k + 1).then_inc(s_dv[G], 1)

    return nc


def _run_cores(nc, in_maps):
    from concourse.bass_utils import run_bass_kernel_spmd

    return run_bass_kernel_spmd(nc, in_maps, list(range(len(in_maps)))).results


def make_in_maps(inputs):
    """Host prep: lse-shift, exp, per-chain gather into step-major streams."""
    import ml_dtypes

    bf16 = ml_dtypes.bfloat16
    x = np.ascontiguousarray(np.asarray(inputs, dtype=np.float32))
    tr = _PROGRAM_CACHE["tr"]
    K, W = K_CHAINS, W_WARM
    D, starts, term_step, snap_step = plan(K, W)
    T = x.shape[1]

    xm = x.max(axis=2, keepdims=True)
    c = (np.log(np.sum(np.exp(x - xm), axis=2, keepdims=True)) + xm).astype(np.float32)
    ex = np.exp(x - c).astype(bf16)  # [B,T,L]

    E = np.exp(tr.astype(np.float32)).astype(bf16)

    posmat = np.empty((K, D), dtype=np.int64)
    for ci in range(K):
        posmat[ci] = np.arange(starts[ci], starts[ci] + D)
    posmat = np.minimum(posmat, T)

    in_maps = []
    for core in range(N_CORES):
        exc = ex[core * BL:(core + 1) * BL]  # [16, T, L]
        pad = np.concatenate(
            [exc, np.ones((BL, 1, L), dtype=bf16)], axis=1
        )  # [16, T+1, L]
        gat = pad[:, posmat, :]  # [16, K, D, L]
        xg = np.ascontiguousarray(
            np.transpose(gat, (3, 2, 1, 0)).reshape(L, D, 16 * K)
        )
        wc = np.ones((L, L + 16 * K), dtype=bf16)
        wc[:, 0:L] = E
        wc[:, L:L + 16] = exc[:, 0, :].T  # chain 0 seeded with e_0
        in_maps.append({"xg": xg, "wconst": np.ascontiguousarray(wc)})
    return in_maps, c


def finish(res, inputs, labels_idx, trans, c):
    """Combine exported chain states host-side in float64."""
    x = np.asarray(inputs)
    lab = np.asarray(labels_idx)
    tr = np.asarray(trans)
    K = K_CHAINS

    lnz = np.empty(B)
    for core in range(N_CORES):
        expb = np.asarray(res[core]["out"]).astype(np.float64)
        g = K // N_GANGS
        off_snap = 16 * K + 16 * g
        terms = [expb[:, 16 * ci:16 * ci + 16] for ci in range(K)]
        terms[0] = expb[:, 16 * K:16 * K + 16]  # chain 0's early terminal
        snaps = {ci: expb[:, off_snap + 16 * ci:off_snap + 16 * ci + 16]
                 for ci in range(1, K)}
        v = np.log(terms[K - 1].sum(axis=0))
        for ci in range(K - 1):
            v += np.log(terms[ci].sum(axis=0)) - np.log(snaps[ci + 1].sum(axis=0))
        lnz[core * BL:(core + 1) * BL] = v

    log_norm = lnz + c.astype(np.float64).sum(axis=1)[:, 0]
    lab64 = lab.astype(np.int64)
    xg = np.take_along_axis(x, lab64[..., None], axis=2)[..., 0].astype(np.float64)
    point = xg.sum(axis=1)
    trans_sc = tr[lab64[:, :-1], lab64[:, 1:]].astype(np.float64).sum(axis=1)
    return (log_norm - point - trans_sc)[:, None].astype(np.float32)


def kernel(inputs, labels_idx, trans):
    if "nc" not in _PROGRAM_CACHE:
        _PROGRAM_CACHE["nc"] = _build_program()
    _PROGRAM_CACHE["tr"] = np.ascontiguousarray(np.asarray(trans, dtype=np.float32))
    nc = _PROGRAM_CACHE["nc"]

    in_maps, c = make_in_maps(inputs)
    res = _run_cores(nc, in_maps)
    return finish(res, inputs, labels_idx, trans, c)
